# revision 20
# baseline (speedup 1.0000x reference)
"""GCN (4-layer, PyG GCNConv semantics) on 8 Trainium2 NeuronCores.

Strategy (graph/data parallel, per sharding hint):
- Nodes partitioned into 8 contiguous blocks of 12500; core c owns block c.
- Per layer l: p = (h @ W_l) * dinv[:, None] computed for owned nodes
  (feature-major hT kept in SBUF; matmul emits node-major tiles).
- AllGather p across cores -> full table in each core's HBM.
- Edges routed to the destination-owning core (host-side), sorted by
  (dst tile of 128, src bucket of 25000), padded to a uniform chunk cap.
- Per dst tile: dma_gather source rows (int16 bucket-local indices),
  segment-sum via indicator matmuls accumulated in PSUM:
      S[p, d] = (dst_local[p] == d);  acc += S.T @ msgs
- Self-loop term: p rows re-read sequentially from the local AG input.
- out = relu(dinv * (acc + p) + b); layer 4 ends with log_softmax.

All per-edge normalization folds into per-node dinv scaling because
msg = h[src]*dinv[src]*dinv[dst] and the self loop contributes dinv^2*h.
"""

import numpy as np

N = 100000
E = 1600000
F_IN = 256
F_HID = 256
F_OUT = 16
NCORES = 8
NPC = N // NCORES          # 12500 nodes per core
P = 128
NT = (NPC + P - 1) // P    # 98 dst tiles per core (last has 84 rows)
NPAD = NT * P              # 12544
NB = 4                     # source buckets (int16 index range)
BSZ = N // NB              # 25000
LAST_ROWS = NPC - (NT - 1) * P  # 84

# message-table dtype for the gather path ("float32" or "bfloat16")
DT_MSG_NAME = "bfloat16"
MSG_BUFS = 12
S_BUFS = 4
P4_COLS = 64               # layer-4 table padded row (64 f32 = 256B)


def _preprocess(edge_index):
    """Route edges to destination-owning cores; build gather indices.

    Returns (idx16, dstf, cap) where per core:
      idx16 [NT*P, NB*IDXCOLS] int16 -- dma_gather index layout, index i of a
          call at [16g + i%16, i//16] (replicated over the 8 groups g).
      dstf [P, NT*NB*CAP] f32 -- local dst id per (chunk, partition), -1 pad.
    """
    src = np.asarray(edge_index[0], dtype=np.int64)
    dst = np.asarray(edge_index[1], dtype=np.int64)

    core = dst // NPC
    dloc = dst - core * NPC
    tile = dloc // P
    buck = src // BSZ
    key = ((core * NT + tile) * NB) + buck
    order = np.argsort(key, kind="stable")
    key_s = key[order]
    src_s = src[order]
    dloc_s = dloc[order]

    ngroups = NCORES * NT * NB
    counts = np.bincount(key_s, minlength=ngroups)
    cap = int((counts.max() + P - 1) // P)
    nidx = cap * P
    idxcols = nidx // 16

    starts = np.zeros(ngroups + 1, dtype=np.int64)
    np.cumsum(counts, out=starts[1:])
    pos = np.arange(E, dtype=np.int64) - starts[key_s]

    chunk = pos // P
    part = pos % P
    i_call = chunk * P + part          # position within the gather call

    # pads stay -1: the dma_gather ucode trims trailing negative indices
    # before descriptor generation, so padding costs no descriptors/bytes.
    idx16 = np.full((NCORES, NT * P, NB * idxcols), -1, dtype=np.int16)
    dstf = np.full((NCORES, P, NT * NB * cap), -1.0, dtype=np.float32)

    c_s = key_s // (NT * NB)
    t_s = (key_s // NB) % NT
    b_s = key_s % NB
    sloc = (src_s - b_s * BSZ).astype(np.int16)
    dtile = (dloc_s - t_s * P).astype(np.float32)

    # idx16 base rows (group 0), then replicate across the 8 groups of 16
    rows = t_s * P + (i_call % 16)
    cols = b_s * idxcols + (i_call // 16)
    idx16[c_s, rows, cols] = sloc
    # replicate within each tile slab: rows [t*P+16g + r] = rows [t*P + r]
    v = idx16.reshape(NCORES, NT, P, NB * idxcols)
    for g in range(1, 8):
        v[:, :, g * 16:(g + 1) * 16, :] = v[:, :, 0:16, :]

    dstf[c_s, part, (t_s * NB + b_s) * cap + chunk] = dtile
    gcnt = counts.reshape(NCORES, NT * NB).astype(np.int32)
    return idx16, dstf, cap, gcnt


def _build_program(cap, dt_msg_name, has_bias, repeats=1):
    import concourse.bass as bass
    import concourse.bacc as bacc
    import concourse.tile as tile
    from concourse import mybir
    from concourse.masks import make_identity

    f32 = mybir.dt.float32
    dt_msg = getattr(mybir.dt, dt_msg_name)
    AFT = mybir.ActivationFunctionType
    ALU = mybir.AluOpType

    nidx = cap * P
    idxcols = nidx // 16
    nch = NB * cap

    nc = bacc.Bacc("TRN2", target_bir_lowering=False, debug=False,
                   num_devices=NCORES, num_swdge_queues=4)

    xT_in = nc.dram_tensor("xT", [F_IN, NPAD], f32, kind="ExternalInput")
    idx_in = nc.dram_tensor("idx16", [NT * P, NB * idxcols], mybir.dt.int16,
                            kind="ExternalInput")
    dst_in = nc.dram_tensor("dstf", [P, NT * nch], dt_msg, kind="ExternalInput")
    dinv_in = nc.dram_tensor("dinvt", [P, NT], f32, kind="ExternalInput")
    gcnt_in = nc.dram_tensor("gcnt", [1, NT * NB], mybir.dt.int32,
                             kind="ExternalInput")
    w_ins = []
    for li in range(4):
        fo = F_OUT if li == 3 else F_HID
        w_ins.append(nc.dram_tensor(f"w{li}", [F_HID, fo], f32,
                                    kind="ExternalInput"))
    b_ins = []
    if has_bias:
        for li in range(4):
            fo = F_OUT if li == 3 else F_HID
            b_ins.append(nc.dram_tensor(f"b{li}", [P, fo], f32,
                                        kind="ExternalInput"))
    out_dram = nc.dram_tensor("out", [NPC, F_OUT], f32, kind="ExternalOutput")

    with tile.TileContext(nc, num_cores=NCORES) as tc:
        with (
            tc.tile_pool(name="const", bufs=1) as cp,
            tc.tile_pool(name="ht", bufs=1) as hp,
            tc.tile_pool(name="idxp", bufs=8) as ixp,
            tc.tile_pool(name="msgs", bufs=MSG_BUFS) as mp,
            tc.tile_pool(name="sel", bufs=S_BUFS) as sp,
            tc.tile_pool(name="work", bufs=3) as wp,
            tc.tile_pool(name="psA", bufs=2, space="PSUM") as psA,
            tc.tile_pool(name="psB", bufs=4, space="PSUM") as psB,
            tc.tile_pool(name="psT", bufs=2, space="PSUM") as psT,
            tc.tile_pool(name="dram", bufs=1, space="DRAM") as dp,
        ):
            agins = [dp.tile([NPC, F_HID], dt_msg, name=f"agin{i}")
                     for i in range(3 * repeats)]
            agouts = [dp.tile([N, F_HID], dt_msg, addr_space="Shared",
                              name=f"agout{i}") for i in range(3 * repeats)]
            agin4s = [dp.tile([NPC, P4_COLS], f32, name=f"agin4_{r}")
                      for r in range(repeats)]
            agout4s = [dp.tile([N, P4_COLS], f32, addr_space="Shared",
                               name=f"agout4_{r}") for r in range(repeats)]

            # constants
            iota_i = cp.tile([P, P], mybir.dt.int32)
            nc.gpsimd.iota(iota_i[:], pattern=[[1, P]], base=0,
                           channel_multiplier=0)
            iota_f = cp.tile([P, P], dt_msg)
            nc.vector.tensor_copy(iota_f[:], iota_i[:])
            ident = cp.tile([P, P], f32)
            make_identity(nc, ident[:])
            ident_b = cp.tile([P, P], dt_msg)
            nc.vector.tensor_copy(ident_b[:], ident[:])

            dinv_sb = cp.tile([P, NT], f32)
            nc.sync.dma_start(out=dinv_sb[:], in_=dinv_in[:])
            gcnt_sb = cp.tile([P, NT * NB], mybir.dt.int32)
            nc.sync.dma_start(out=gcnt_sb[0:1, :], in_=gcnt_in[:])
            dstf_sb = cp.tile([P, NT * nch], dt_msg)
            nc.sync.dma_start(out=dstf_sb[:], in_=dst_in[:])

            w_sb = []
            for li in range(4):
                fo = F_OUT if li == 3 else F_HID
                pair = []
                for k in range(2):
                    w = cp.tile([P, fo], f32, name=f"w{li}_{k}")
                    nc.sync.dma_start(out=w[:], in_=w_ins[li][k * P:(k + 1) * P, :])
                    pair.append(w)
                w_sb.append(pair)
            b_sb = []
            if has_bias:
                for li in range(4):
                    fo = F_OUT if li == 3 else F_HID
                    bt = cp.tile([P, fo], f32, name=f"b{li}_sb")
                    nc.sync.dma_start(out=bt[:], in_=b_ins[li][:])
                    b_sb.append(bt)

            msgs_seen = [0]
            cregs = [nc.gpsimd.alloc_register(f"cnt{i}") for i in range(8)]
            creg_i = [0]
            hT0 = hp.tile([P, NPAD], f32)
            hT1 = hp.tile([P, NPAD], f32)
            nc.sync.dma_start(out=hT0[:], in_=xT_in[0:P, :])
            nc.sync.dma_start(out=hT1[:], in_=xT_in[P:2 * P, :])

            for rep in range(repeats):
              for li in range(4):
                last = li == 3
                fo = F_OUT if last else F_HID
                l_agin = agin4s[rep] if last else agins[rep * 3 + li]
                l_agout = agout4s[rep] if last else agouts[rep * 3 + li]
                l_dt = f32 if last else dt_msg
                elem = P4_COLS if last else F_HID

                def phase_a_tile(li_a, t, agin_a):
                    la = li_a == 3
                    fo_a = F_OUT if la else F_HID
                    dt_a = f32 if la else dt_msg
                    rows_a = LAST_ROWS if t == NT - 1 else P
                    csl = slice(t * P, (t + 1) * P)
                    pa = psA.tile([P, fo_a], f32, tag="pa")
                    nc.tensor.matmul(pa[:], lhsT=hT0[:, csl],
                                     rhs=w_sb[li_a][0][:],
                                     start=True, stop=False)
                    nc.tensor.matmul(pa[:], lhsT=hT1[:, csl],
                                     rhs=w_sb[li_a][1][:],
                                     start=False, stop=True)
                    ps = wp.tile([P, F_HID], dt_a, tag="ps",
                                 padded_shape=[P, F_HID])
                    nc.scalar.activation(ps[:, 0:fo_a], pa[:], AFT.Copy,
                                         scale=dinv_sb[:, t:t + 1])
                    nc.sync.dma_start(out=agin_a[t * P:t * P + rows_a, 0:fo_a],
                                      in_=ps[:rows_a, 0:fo_a])

                # ---- phase A (only layer of this rep's start; later layers
                # are fused into the previous layer's phase B) ----
                if li == 0:
                    for t in range(NT):
                        phase_a_tile(0, t, l_agin)

                # ---- AllGather ----
                nc.gpsimd.collective_compute(
                    "AllGather", mybir.AluOpType.bypass,
                    replica_groups=[list(range(NCORES))],
                    ins=[l_agin[:]], outs=[l_agout[:]],
                )

                # ---- phase B: segment sum + tail ----
                for t in range(NT):
                    rows = LAST_ROWS if t == NT - 1 else P
                    idxt = ixp.tile([P, NB * idxcols], mybir.dt.int16, tag="idxt")
                    nc.sync.dma_start(out=idxt[:],
                                      in_=idx_in[t * P:(t + 1) * P, :])
                    acc = psB.tile([P, fo], f32, tag="acc")
                    # all-of-tile indicator matrices in one DVE op:
                    # S_all[p, ch, d] = (dstf[p, t*nch+ch] == d)
                    S_all = sp.tile([P, nch, P], l_dt, tag="S")
                    nc.vector.tensor_tensor(
                        out=S_all[:],
                        in0=dstf_sb[:, t * nch:(t + 1) * nch, None]
                            .to_broadcast([P, nch, P]),
                        in1=iota_f[:, None, :].to_broadcast([P, nch, P]),
                        op=ALU.is_equal,
                    )
                    for b in range(NB):
                        msgs = mp.tile([P, cap, elem], l_dt, tag="msgs")
                        if msgs_seen[0] < MSG_BUFS:
                            msgs_seen[0] += 1
                            nc.vector.memset(msgs[:], 0)
                        creg = cregs[creg_i[0] % 8]
                        creg_i[0] += 1
                        nc.gpsimd.reg_load(
                            creg, gcnt_sb[0:1, t * NB + b:t * NB + b + 1])
                        nc.gpsimd.dma_gather(
                            msgs[:],
                            l_agout[b * BSZ:(b + 1) * BSZ, :],
                            idxt[:, b * idxcols:(b + 1) * idxcols],
                            nidx, creg, elem,
                            queue_num=b,
                        )
                        for j in range(cap):
                            ch = b * cap + j
                            nc.tensor.matmul(
                                acc[:], lhsT=S_all[:, ch, :],
                                rhs=msgs[:, j, 0:fo],
                                start=(b == 0 and j == 0),
                                stop=False,
                            )
                    pself = wp.tile([P, fo], l_dt, tag="pself")
                    nc.sync.dma_start(out=pself[:rows, :],
                                      in_=l_agin[t * P:t * P + rows, 0:fo])
                    nc.tensor.matmul(
                        acc[:], lhsT=(ident if last else ident_b)[:],
                        rhs=pself[:, :], start=False, stop=True)
                    hpre = acc
                    if not last:
                        h = wp.tile([P, F_HID], f32, tag="h")
                        if has_bias:
                            hs = wp.tile([P, F_HID], f32, tag="hs")
                            nc.scalar.activation(hs[:rows, :], hpre[:rows, :],
                                                 AFT.Copy,
                                                 scale=dinv_sb[:rows, t:t + 1])
                            hb = wp.tile([P, F_HID], f32, tag="hb")
                            nc.vector.tensor_tensor(out=hb[:rows, :],
                                                    in0=hs[:rows, :],
                                                    in1=b_sb[li][:rows, :],
                                                    op=ALU.add)
                            nc.scalar.activation(h[:rows, :], hb[:rows, :],
                                                 AFT.Relu)
                        else:
                            nc.scalar.activation(h[:rows, :], hpre[:rows, :],
                                                 AFT.Relu,
                                                 scale=dinv_sb[:rows, t:t + 1])
                        # transpose h into hT for next layer's matmul
                        for k in range(2):
                            tp = psT.tile([P, P], f32, tag="tp")
                            nc.tensor.transpose(tp[:], h[:, k * P:(k + 1) * P],
                                                ident[:])
                            hT = hT0 if k == 0 else hT1
                            nc.vector.tensor_copy(hT[:, t * P:(t + 1) * P],
                                                  tp[:])
                        # fused phase A of the next layer for this tile
                        nli = li + 1
                        nlast = nli == 3
                        phase_a_tile(nli, t,
                                     (agin4s[rep] if nlast
                                      else agins[rep * 3 + nli]))
                    else:
                        t4 = wp.tile([P, F_OUT], f32, tag="t4")
                        nc.scalar.activation(t4[:rows, :], hpre[:rows, :],
                                             AFT.Copy,
                                             scale=dinv_sb[:rows, t:t + 1])
                        if has_bias:
                            t4b = wp.tile([P, F_OUT], f32, tag="t4b")
                            nc.vector.tensor_tensor(out=t4b[:rows, :],
                                                    in0=t4[:rows, :],
                                                    in1=b_sb[3][:rows, :],
                                                    op=ALU.add)
                            t4 = t4b
                        m = wp.tile([P, 1], f32, tag="m")
                        nc.vector.reduce_max(m[:rows, :], t4[:rows, :],
                                             axis=mybir.AxisListType.X)
                        mneg = wp.tile([P, 1], f32, tag="mneg")
                        nc.vector.tensor_scalar_mul(mneg[:rows, :], m[:rows, :],
                                                    -1.0)
                        ex = wp.tile([P, F_OUT], f32, tag="ex")
                        se = wp.tile([P, 1], f32, tag="se")
                        nc.scalar.activation(ex[:rows, :], t4[:rows, :], AFT.Exp,
                                             bias=mneg[:rows, :],
                                             accum_out=se[:rows, :])
                        lse = wp.tile([P, 1], f32, tag="lse")
                        nc.scalar.activation(lse[:rows, :], se[:rows, :], AFT.Ln)
                        ot = wp.tile([P, F_OUT], f32, tag="ot")
                        nc.vector.tensor_scalar(
                            out=ot[:rows, :], in0=t4[:rows, :],
                            scalar1=mneg[:rows, :1], scalar2=lse[:rows, :1],
                            op0=ALU.add, op1=ALU.subtract)
                        nc.sync.dma_start(out=out_dram[t * P:t * P + rows, :],
                                          in_=ot[:rows, :])

    nc.compile()
    return nc


_PROGRAM_CACHE = {}
_LAST_IN_MAPS = None


def kernel(**inputs):
    from concourse.bass_utils import run_bass_kernel_spmd

    x = np.asarray(inputs["x"], dtype=np.float32)
    edge_index = np.asarray(inputs["edge_index"])
    Ws = [np.asarray(inputs[f"W{i}"], dtype=np.float32) for i in range(1, 5)]
    bs = [np.asarray(inputs[f"b{i}"], dtype=np.float32) for i in range(1, 5)]

    dst = edge_index[1].astype(np.int64)
    deg = np.bincount(dst, minlength=N).astype(np.float32) + 1.0
    dinv = 1.0 / np.sqrt(deg)

    idx16, dstf, cap, gcnt = _preprocess(edge_index)
    has_bias = any(np.any(b != 0) for b in bs)

    ck = (cap, DT_MSG_NAME, has_bias)
    if ck not in _PROGRAM_CACHE:
        _PROGRAM_CACHE[ck] = _build_program(cap, DT_MSG_NAME, has_bias)
    nc = _PROGRAM_CACHE[ck]

    in_maps = []
    for c in range(NCORES):
        xc = x[c * NPC:(c + 1) * NPC]
        xT = np.zeros((F_IN, NPAD), dtype=np.float32)
        xT[:, :NPC] = xc.T
        dvc = dinv[c * NPC:(c + 1) * NPC]
        tmp = np.ones((NT, P), dtype=np.float32)
        tmp.reshape(-1)[:NPC] = dvc  # [t, p] row-major = node order
        dinvt = np.ascontiguousarray(tmp.T)
        import ml_dtypes
        m = {
            "xT": xT,
            "idx16": idx16[c],
            "dstf": dstf[c].astype(ml_dtypes.bfloat16)
                    if DT_MSG_NAME == "bfloat16" else dstf[c],
            "dinvt": dinvt,
            "gcnt": gcnt[c].reshape(1, NT * NB),
            "w0": Ws[0], "w1": Ws[1], "w2": Ws[2],
            "w3": np.ascontiguousarray(Ws[3]),
        }
        if has_bias:
            for li in range(4):
                fo = F_OUT if li == 3 else F_HID
                m[f"b{li}"] = np.tile(bs[li].reshape(1, fo), (P, 1)).astype(np.float32)
        in_maps.append(m)

    global _LAST_IN_MAPS
    _LAST_IN_MAPS = in_maps
    res = run_bass_kernel_spmd(nc, in_maps, list(range(NCORES)))
    out = np.concatenate([res.results[c]["out"] for c in range(NCORES)], axis=0)
    return out.astype(np.float32)
